# revision 1
# baseline (speedup 1.0000x reference)
import sys

import numpy as np

for _p in ("/opt/trn_rl_repo",):
    if _p not in sys.path:
        sys.path.insert(0, _p)

B = 4096
M = 8192
EMB = 64
K = 4
TAU = 0.3
NCORES = 8
BLOC = B // NCORES  # 512 batch rows per core
P = 128             # batch rows per tile
NBT = BLOC // P     # 4 tiles per core
CM = 4096           # anchors per m-chunk
NCH = M // CM       # 2 chunks

_CACHE = {}


def _build(debug=False, iters=1, no_gather=False, no_smalls=False, no_topk=False, no_combine=False, no_squares=False):
    from contextlib import ExitStack

    import concourse.bacc as bacc
    import concourse.bass as bass
    import concourse.mybir as mybir
    import concourse.tile as tile
    from concourse.masks import make_identity

    f32 = mybir.dt.float32
    u32 = mybir.dt.uint32
    AF = mybir.ActivationFunctionType
    OP = mybir.AluOpType
    AX = mybir.AxisListType

    nc = bacc.Bacc()
    nodes_h = nc.declare_dram_parameter("nodes", [BLOC, 2, 2], f32, isOutput=False)
    ancS_h = nc.declare_dram_parameter("ancS", [BLOC, M, 2], f32, isOutput=False)
    ancL_h = nc.declare_dram_parameter("ancL", [BLOC, M, 2], f32, isOutput=False)
    W1_h = nc.declare_dram_parameter("W1", [EMB, 2], f32, isOutput=False)
    b1_h = nc.declare_dram_parameter("b1", [EMB], f32, isOutput=False)
    W2_h = nc.declare_dram_parameter("W2", [EMB, EMB], f32, isOutput=False)
    b2_h = nc.declare_dram_parameter("b2", [EMB], f32, isOutput=False)
    out_h = nc.declare_dram_parameter("out", [BLOC, 2 * EMB], f32, isOutput=True)
    if debug:
        dbgf_h = nc.declare_dram_parameter("dbgf", [BLOC, 2, 16], f32, isOutput=True)
        dbgi_h = nc.declare_dram_parameter("dbgi", [BLOC, 2, 8], u32, isOutput=True)

    with ExitStack() as ctx:
        tc = ctx.enter_context(tile.TileContext(nc))
        const = ctx.enter_context(tc.tile_pool(name="const", bufs=1))
        a_pool = ctx.enter_context(tc.tile_pool(name="a", bufs=2))
        sq_pool = ctx.enter_context(tc.tile_pool(name="sq", bufs=2))
        mneg_pool = ctx.enter_context(tc.tile_pool(name="mneg", bufs=2))
        small = ctx.enter_context(tc.tile_pool(name="small", bufs=2))
        mlp = ctx.enter_context(tc.tile_pool(name="mlp", bufs=2))
        psum = ctx.enter_context(tc.tile_pool(name="psum", bufs=1, space="PSUM"))

        ident = const.tile([P, P], f32)
        make_identity(nc, ident[:])

        # Warm-up Gelu: anchors the ACT table chooser on gelu_and_others
        # (gelu/square/tanh/copy) so the kernel needs a single table load.
        dummy = const.tile([1, 1], f32)
        nc.scalar.activation(dummy[:], dummy[:], AF.Gelu, bias=0.0, scale=1.0)

        w1t = const.tile([2, EMB], f32)  # w1t[c, e] = W1[e, c]
        nc.sync.dma_start(out=w1t[:], in_=W1_h[:].rearrange("e c -> c e"))
        w2t = const.tile([EMB, EMB], f32)  # w2t[e, f] = W2[f, e]
        nc.sync.dma_start(out=w2t[:], in_=W2_h[:].rearrange("f e -> e f"))
        b1c = const.tile([EMB, 1], f32)
        nc.sync.dma_start(out=b1c[:], in_=b1_h[:].rearrange("(e u) -> e u", u=1))
        b2c = const.tile([EMB, 1], f32)
        nc.sync.dma_start(out=b2c[:], in_=b2_h[:].rearrange("(e u) -> e u", u=1))

        def _body():
          for bt in range(NBT):
            rows = slice(bt * P, (bt + 1) * P)

            nodes_t = small.tile([P, 4], f32)
            nc.scalar.dma_start(
                out=nodes_t[:], in_=nodes_h[rows, :, :].rearrange("p a c -> p (a c)")
            )
            negn = small.tile([P, 4], f32)
            nc.gpsimd.tensor_scalar(
                out=negn[:], in0=nodes_t[:], scalar1=-1.0, scalar2=None, op0=OP.mult
            )

            rowbase = small.tile([P, 1], u32)
            nc.gpsimd.iota(
                rowbase[:], pattern=[[0, 1]], base=bt * P * M, channel_multiplier=M
            )

            for br in range(2):
                anc_h = ancS_h if br == 0 else ancL_h
                mneg = mneg_pool.tile([P, M], f32)
                for chk in range(NCH):
                    a_t = a_pool.tile([P, 2 * CM], f32)
                    nc.gpsimd.dma_start(
                        out=a_t[:],
                        in_=anc_h[rows, chk * CM:(chk + 1) * CM, :].rearrange(
                            "p m c -> p (m c)"
                        ),
                    )
                    av = a_t[:].rearrange("p (m c) -> p m c", c=2)
                    msl = mneg[:, chk * CM:(chk + 1) * CM]
                    v2 = sq_pool.tile([P, CM], f32)
                    if no_squares:
                        continue
                    # u^2 straight into the mneg slice; v^2 to scratch
                    nc.scalar.activation(
                        msl, av[:, :, 0], AF.Square,
                        bias=negn[:, 2 * br:2 * br + 1], scale=1.0,
                    )
                    nc.scalar.activation(
                        v2[:], av[:, :, 1], AF.Square,
                        bias=negn[:, 2 * br + 1:2 * br + 2], scale=1.0,
                    )
                    if no_combine:
                        continue
                    # mneg = (v2 * -1) - u2 = -d2, one fused DVE pass in place.
                    # (GPSIMD elementwise measured ~10x slower here - keep off it.)
                    nc.vector.scalar_tensor_tensor(
                        out=msl, in0=v2[:], scalar=-1.0, in1=msl,
                        op0=OP.mult, op1=OP.subtract,
                    )

                if no_topk:
                    continue
                vals8 = small.tile([P, 8], f32)
                nc.vector.max(out=vals8[:], in_=mneg[:])
                idx8 = small.tile([P, 8], u32)
                nc.vector.max_index(out=idx8[:], in_max=vals8[:], in_values=mneg[:])
                if no_smalls:
                    continue

                gidx = small.tile([P, K], u32)
                nc.vector.tensor_tensor(
                    out=gidx[:], in0=idx8[:, 0:K],
                    in1=rowbase[:].to_broadcast([P, K]), op=OP.add,
                )

                # indirect DMA honors one offset per partition: gather the K
                # neighbors' (x, y) pairs with K separate calls
                comb = small.tile([P, 2 * K], f32)
                if no_gather:
                    nc.vector.memset(comb[:], 0.5)
                else:
                    for k in range(K):
                        nc.gpsimd.indirect_dma_start(
                            out=comb[:, 2 * k:2 * k + 2],
                            out_offset=None,
                            in_=anc_h[:].rearrange("b m c -> (b m) c"),
                            in_offset=bass.IndirectOffsetOnAxis(ap=gidx[:, k:k + 1], axis=0),
                        )

                # w = softmax(d2_topk / tau); vals8[:, :K] = -d2 (descending)
                t4 = small.tile([P, K], f32)
                nc.vector.tensor_scalar(
                    out=t4[:], in0=vals8[:, 0:K], scalar1=float(-1.0 / TAU),
                    scalar2=None, op0=OP.mult,
                )
                # vals8 is sorted descending, so t4's max is at column K-1:
                # nrmaxh = -max(t4)/2 = vals8[:, K-1] / (2*tau)
                nrmaxh = small.tile([P, 1], f32)
                nc.vector.tensor_scalar(
                    out=nrmaxh[:], in0=vals8[:, K - 1:K], scalar1=float(0.5 / TAU),
                    scalar2=None, op0=OP.mult,
                )
                # exp(x) = 2/(1 - tanh(x/2)) - 1 with x = t4 - max(t4);
                # keeps ACT inside the gelu_and_others table set (no exp there).
                th = small.tile([P, K], f32)
                nc.scalar.activation(th[:], t4[:], AF.Tanh, bias=nrmaxh[:, 0:1], scale=0.5)
                denom = small.tile([P, K], f32)
                nc.vector.tensor_scalar(
                    out=denom[:], in0=th[:], scalar1=-1.0, scalar2=1.0,
                    op0=OP.mult, op1=OP.add,
                )
                rden = small.tile([P, K], f32)
                nc.vector.reciprocal(rden[:], denom[:])
                e4 = small.tile([P, K], f32)
                nc.vector.tensor_scalar(
                    out=e4[:], in0=rden[:], scalar1=2.0, scalar2=-1.0,
                    op0=OP.mult, op1=OP.add,
                )
                ssum = small.tile([P, 1], f32)
                nc.vector.reduce_sum(out=ssum[:], in_=e4[:], axis=AX.X)
                rinv = small.tile([P, 1], f32)
                nc.vector.reciprocal(rinv[:], ssum[:])
                wnorm = small.tile([P, K], f32)
                nc.vector.tensor_scalar(
                    out=wnorm[:], in0=e4[:], scalar1=rinv[:, 0:1],
                    scalar2=None, op0=OP.mult,
                )

                # [P, 2] slices -> [2, P] each, packed into [2, K*P]:
                # columns k*P..(k+1)*P hold (Ax, Ay) of neighbor k for all b
                tp_ps = psum.tile([2, K * P], f32)
                for k in range(K):
                    nc.tensor.transpose(
                        out=tp_ps[:, k * P:(k + 1) * P],
                        in_=comb[:, 2 * k:2 * k + 2], identity=ident[:],
                    )
                rhs_all = mlp.tile([2, K * P], f32)
                nc.vector.tensor_copy(rhs_all[:], tp_ps[:])

                hp = psum.tile([EMB, K * P], f32)
                nc.tensor.matmul(
                    out=hp[:], lhsT=w1t[:], rhs=rhs_all[:], start=True, stop=True
                )
                h1 = mlp.tile([EMB, K * P], f32)
                nc.scalar.activation(h1[:], hp[:], AF.Gelu, bias=b1c[:, 0:1], scale=1.0)
                h2p = psum.tile([EMB, K * P], f32)
                nc.tensor.matmul(
                    out=h2p[:], lhsT=w2t[:], rhs=h1[:], start=True, stop=True
                )
                h2 = mlp.tile([EMB, K * P], f32)
                nc.scalar.activation(h2[:], h2p[:], AF.Gelu, bias=b2c[:, 0:1], scale=1.0)

                # weighted sum over neighbors, back in b-on-partitions layout.
                # memset first so the osb-slot WAR wait (vs the old output DMA)
                # rides on the memset, keeping the stt ops under the sync cap.
                osb = small.tile([P, EMB], f32)
                nc.vector.memset(osb[:], 0.0)
                for k in range(K):
                    h2tp = psum.tile([P, EMB], f32)
                    nc.tensor.transpose(
                        out=h2tp[:], in_=h2[:, k * P:(k + 1) * P],
                        identity=ident[0:EMB, 0:EMB],
                    )
                    nc.vector.scalar_tensor_tensor(
                        out=osb[:], in0=h2tp[:], scalar=wnorm[:, k:k + 1],
                        in1=osb[:], op0=OP.mult, op1=OP.add,
                    )
                nc.scalar.dma_start(
                    out=out_h[rows, br * EMB:(br + 1) * EMB], in_=osb[:]
                )
                if debug:
                    dbf = small.tile([P, 16], f32)
                    nc.vector.tensor_copy(dbf[:, 0:4], vals8[:, 0:4])
                    nc.vector.tensor_copy(dbf[:, 4:12], comb[:])
                    nc.vector.tensor_copy(dbf[:, 12:16], wnorm[:])
                    nc.sync.dma_start(out=dbgf_h[rows, br, :], in_=dbf[:])
                    dbi = small.tile([P, 8], u32)
                    nc.vector.tensor_copy(dbi[:, 0:4], idx8[:, 0:4])
                    nc.vector.tensor_copy(dbi[:, 4:8], gidx[:])
                    nc.sync.dma_start(out=dbgi_h[rows, br, :], in_=dbi[:])

        if iters > 1:
            with tc.For_i(0, iters, 1):
                _body()
        else:
            _body()
    return nc


def _get_nc(debug=False):
    key = ("nc", debug)
    if key not in _CACHE:
        nc = _build(debug)
        nc.finalize()  # runs the Bacc passes (event sems, reg alloc, table loads)
        _CACHE[key] = nc
    return _CACHE[key]


def _make_in_maps(inputs):
    nodes = np.asarray(inputs["nodes_2x2"], dtype=np.float32)
    ancS = np.asarray(inputs["ancS"], dtype=np.float32)
    ancL = np.asarray(inputs["ancL"], dtype=np.float32)
    W1 = np.asarray(inputs["W1"], dtype=np.float32)
    b1 = np.asarray(inputs["b1"], dtype=np.float32)
    W2 = np.asarray(inputs["W2"], dtype=np.float32)
    b2 = np.asarray(inputs["b2"], dtype=np.float32)
    in_maps = []
    for c in range(NCORES):
        sl = slice(c * BLOC, (c + 1) * BLOC)
        in_maps.append(
            {
                "nodes": np.ascontiguousarray(nodes[sl]),
                "ancS": np.ascontiguousarray(ancS[sl]),
                "ancL": np.ascontiguousarray(ancL[sl]),
                "W1": W1,
                "b1": b1,
                "W2": W2,
                "b2": b2,
            }
        )
    return in_maps


def _run(in_maps, trace=False, debug=False):
    from concourse.bass_utils import run_bass_kernel_spmd

    nc = _get_nc(debug)
    return run_bass_kernel_spmd(nc, in_maps, core_ids=list(range(NCORES)), trace=trace)


def kernel(**inputs):
    in_maps = _make_in_maps(inputs)
    res = _run(in_maps).results
    out = np.concatenate([res[c]["out"] for c in range(NCORES)], axis=0)
    return out[:, :EMB].copy(), out[:, EMB:].copy()



# revision 18
# speedup vs baseline: 27418.3900x; 27418.3900x over previous
import sys

import numpy as np

for _p in ("/opt/trn_rl_repo",):
    if _p not in sys.path:
        sys.path.insert(0, _p)

B = 4096
M = 8192
EMB = 64
K = 4
TAU = 0.3
NCORES = 8
BLOC = B // NCORES  # 512 batch rows per core
P = 128             # batch rows per tile
NBT = BLOC // P     # 4 tiles per core
CM = 4096           # anchors per m-chunk
NCH = M // CM       # 2 chunks
SLOT = 16           # anchors per top-k slot
NSLOT = M // SLOT   # 512 slots per row
NSC = CM // SLOT    # 256 slots per chunk
NCAND = 8 * SLOT    # 128 rescan candidates (top-8 slots)

_CACHE = {}


def _build(debug=False, variant=""):
    no_negate = "no_negate" in variant
    no_slotred = "no_slotred" in variant
    no_rescan = "no_rescan" in variant
    no_iota128 = "no_iota128" in variant
    no_ttr = "no_ttr" in variant
    oh_only = "oh_only" in variant
    ttr_contig = "ttr_contig" in variant
    from contextlib import ExitStack

    import concourse.bacc as bacc
    import concourse.bass as bass
    import concourse.mybir as mybir
    import concourse.tile as tile
    from concourse.masks import make_identity

    f32 = mybir.dt.float32
    bf16 = mybir.dt.bfloat16
    u32 = mybir.dt.uint32
    AF = mybir.ActivationFunctionType
    OP = mybir.AluOpType
    AX = mybir.AxisListType

    nc = bacc.Bacc()
    nodes_h = nc.declare_dram_parameter("nodes", [BLOC, 2, 2], f32, isOutput=False)
    ancS_h = nc.declare_dram_parameter("ancS", [BLOC, M, 2], f32, isOutput=False)
    ancL_h = nc.declare_dram_parameter("ancL", [BLOC, M, 2], f32, isOutput=False)
    W1_h = nc.declare_dram_parameter("W1", [EMB, 2], f32, isOutput=False)
    b1_h = nc.declare_dram_parameter("b1", [EMB], f32, isOutput=False)
    W2_h = nc.declare_dram_parameter("W2", [EMB, EMB], f32, isOutput=False)
    b2_h = nc.declare_dram_parameter("b2", [EMB], f32, isOutput=False)
    out_h = nc.declare_dram_parameter("out", [BLOC, 2 * EMB], f32, isOutput=True)
    if debug:
        dbgf_h = nc.declare_dram_parameter("dbgf", [BLOC, 2, 32], f32, isOutput=True)
        dbgi_h = nc.declare_dram_parameter("dbgi", [BLOC, 2, 16], u32, isOutput=True)

    with ExitStack() as ctx:
        tc = ctx.enter_context(tile.TileContext(nc))
        const = ctx.enter_context(tc.tile_pool(name="const", bufs=1))
        a_pool = ctx.enter_context(tc.tile_pool(name="a", bufs=3))
        sq_pool = ctx.enter_context(tc.tile_pool(name="sq", bufs=2))
        d2_pool = ctx.enter_context(tc.tile_pool(name="d2", bufs=2))
        slot_pool = ctx.enter_context(tc.tile_pool(name="slot", bufs=2))
        cand_pool = ctx.enter_context(tc.tile_pool(name="cand", bufs=2))
        small = ctx.enter_context(tc.tile_pool(name="small", bufs=2))
        mlp = ctx.enter_context(tc.tile_pool(name="mlp", bufs=2))
        psum = ctx.enter_context(tc.tile_pool(name="psum", bufs=1, space="PSUM"))

        ident = const.tile([P, P], f32)
        make_identity(nc, ident[:])

        # Warm-up Gelu: anchors the ACT table chooser on gelu_and_others
        # (gelu/square/tanh/copy) so the kernel needs a single table load.
        dummy = const.tile([1, 1], f32)
        nc.vector.memset(dummy[:], 0.0)
        nc.scalar.activation(dummy[:], dummy[:], AF.Gelu, bias=0.0, scale=1.0)

        w1t = const.tile([2, EMB], f32)  # w1t[c, e] = W1[e, c]
        nc.sync.dma_start(out=w1t[:], in_=W1_h[:].rearrange("e c -> c e"))
        w2t = const.tile([EMB, EMB], f32)  # w2t[e, f] = W2[f, e]
        nc.sync.dma_start(out=w2t[:], in_=W2_h[:].rearrange("f e -> e f"))
        b1c = const.tile([EMB, 1], f32)
        nc.sync.dma_start(out=b1c[:], in_=b1_h[:].rearrange("(e u) -> e u", u=1))
        b2c = const.tile([EMB, 1], f32)
        nc.sync.dma_start(out=b2c[:], in_=b2_h[:].rearrange("(e u) -> e u", u=1))

        # iota over candidate positions (f32, for exact is_equal onehots)
        # [0..127, 0..127] as f32: onehot domain over the (u | v) concat layout
        iotau = const.tile([P, 2 * NCAND], u32)
        nc.gpsimd.iota(iotau[:], pattern=[[1, 2 * NCAND]], base=0, channel_multiplier=0)
        iotam = const.tile([P, 2 * NCAND], u32)
        nc.vector.tensor_scalar(
            out=iotam[:], in0=iotau[:], scalar1=NCAND - 1, scalar2=None,
            op0=OP.bitwise_and,
        )
        iotaf = const.tile([P, 2 * NCAND], f32)
        nc.vector.tensor_copy(iotaf[:], iotam[:])

        # flat view of anchors for the slot rescan: row r = b*NSLOT + s holds
        # the 16 (x, y) pairs of slot s of batch-row b (32 f32 = 128B)
        ancS_slots = ancS_h[:].rearrange("b (s j) c -> (b s) (j c)", j=SLOT)
        ancL_slots = ancL_h[:].rearrange("b (s j) c -> (b s) (j c)", j=SLOT)

        for bt in range(NBT):
            rows = slice(bt * P, (bt + 1) * P)

            nodes_t = small.tile([P, 4], f32)
            nc.scalar.dma_start(
                out=nodes_t[:], in_=nodes_h[rows, :, :].rearrange("p a c -> p (a c)")
            )
            negn = small.tile([P, 4], f32)
            nc.gpsimd.tensor_scalar(
                out=negn[:], in0=nodes_t[:], scalar1=-1.0, scalar2=None, op0=OP.mult
            )

            # rowbase over the slot-flattened anchor view
            rowbase = small.tile([P, 1], u32)
            nc.gpsimd.iota(
                rowbase[:], pattern=[[0, 1]], base=bt * P * NSLOT,
                channel_multiplier=NSLOT,
            )

            for br in range(2):
                anc_slots = ancS_slots if br == 0 else ancL_slots
                nslot = slot_pool.tile([P, NSLOT], f32)  # -min(d2) per slot
                for chk in range(NCH):
                    a_t = a_pool.tile([P, 2 * CM], f32)
                    nc.gpsimd.dma_start(
                        out=a_t[:],
                        in_=(ancS_h if br == 0 else ancL_h)[
                            rows, chk * CM:(chk + 1) * CM, :
                        ].rearrange("p m c -> p (m c)"),
                    )
                    av = a_t[:].rearrange("p (m c) -> p m c", c=2)
                    u2 = sq_pool.tile([P, CM], bf16)
                    v2 = sq_pool.tile([P, CM], bf16)
                    nc.scalar.activation(
                        u2[:], av[:, :, 0], AF.Square,
                        bias=negn[:, 2 * br:2 * br + 1], scale=1.0,
                    )
                    nc.scalar.activation(
                        v2[:], av[:, :, 1], AF.Square,
                        bias=negn[:, 2 * br + 1:2 * br + 2], scale=1.0,
                    )
                    d2c = d2_pool.tile([P, CM], bf16)
                    nc.vector.tensor_tensor(
                        out=d2c[:], in0=u2[:], in1=v2[:], op=OP.add
                    )
                    # per-slot min over groups of 16, negated so max8 ranks
                    # nearest slots first
                    if no_slotred:
                        nc.vector.memset(nslot[:, chk * NSC:(chk + 1) * NSC], 0.0)
                    elif no_negate:
                        nc.vector.tensor_reduce(
                            out=nslot[:, chk * NSC:(chk + 1) * NSC],
                            in_=d2c[:].rearrange("p (s j) -> p s j", j=SLOT),
                            axis=AX.X, op=OP.min,
                        )
                    else:
                        nc.vector.tensor_reduce(
                            out=nslot[:, chk * NSC:(chk + 1) * NSC],
                            in_=d2c[:].rearrange("p (s j) -> p s j", j=SLOT),
                            axis=AX.X, op=OP.min, negate=True,
                        )

                svals8 = small.tile([P, 8], f32)
                nc.vector.max(out=svals8[:], in_=nslot[:])
                sidx8 = small.tile([P, 8], u32)
                nc.vector.max_index(out=sidx8[:], in_max=svals8[:], in_values=nslot[:])

                # offsets into the (b s) axis of the slot-flattened anchors
                soff = small.tile([P, 8], u32)
                nc.vector.tensor_tensor(
                    out=soff[:], in0=sidx8[:],
                    in1=rowbase[:].to_broadcast([P, 8]), op=OP.add,
                )

                # rescan: pull the top-8 slots' raw coords (16 pairs each)
                cand = cand_pool.tile([P, 2 * NCAND], f32)
                if no_rescan:
                    nc.vector.memset(cand[:], 0.5)
                else:
                    for r in range(8):
                        nc.gpsimd.indirect_dma_start(
                            out=cand[:, 32 * r:32 * (r + 1)],
                            out_offset=None,
                            in_=anc_slots,
                            in_offset=bass.IndirectOffsetOnAxis(
                                ap=soff[:, r:r + 1], axis=0
                            ),
                        )

                cv = cand[:].rearrange("p (m c) -> p m c", c=2)
                cu2 = cand_pool.tile([P, NCAND], f32)
                cv2 = cand_pool.tile([P, NCAND], f32)
                nc.scalar.activation(
                    cu2[:], cv[:, :, 0], AF.Square,
                    bias=negn[:, 2 * br:2 * br + 1], scale=1.0,
                )
                nc.scalar.activation(
                    cv2[:], cv[:, :, 1], AF.Square,
                    bias=negn[:, 2 * br + 1:2 * br + 2], scale=1.0,
                )
                ncd = cand_pool.tile([P, NCAND], f32)  # exact -d2 of candidates
                nc.vector.scalar_tensor_tensor(
                    out=ncd[:], in0=cu2[:], scalar=-1.0, in1=cv2[:],
                    op0=OP.mult, op1=OP.subtract,
                )

                cvals8 = small.tile([P, 8], f32)
                nc.vector.max(out=cvals8[:], in_=ncd[:])
                cpos8 = small.tile([P, 8], u32)
                nc.vector.max_index(out=cpos8[:], in_max=cvals8[:], in_values=ncd[:])
                cposf = small.tile([P, K], f32)
                nc.vector.tensor_copy(cposf[:], cpos8[:, 0:K])

                # extract the top-4 coords from cand: deinterleave to (u | v)
                # concat, then per-k onehot dot-products (tt-mult + reduce)
                canduv = cand_pool.tile([P, 2 * NCAND], f32)
                nc.scalar.activation(
                    canduv[:, 0:NCAND], cv[:, :, 0], AF.Copy, bias=0.0, scale=1.0
                )
                nc.scalar.activation(
                    canduv[:, NCAND:2 * NCAND], cv[:, :, 1], AF.Copy,
                    bias=0.0, scale=1.0,
                )
                comb = small.tile([P, 2 * K], f32)
                oh = cand_pool.tile([P, 2 * NCAND], f32)
                prod = cand_pool.tile([P, 2 * NCAND], f32)
                for k in range(K):
                    nc.vector.tensor_scalar(
                        out=oh[:], in0=iotaf[:], scalar1=cposf[:, k:k + 1],
                        scalar2=None, op0=OP.is_equal,
                    )
                    nc.vector.tensor_tensor(
                        out=prod[:], in0=oh[:], in1=canduv[:], op=OP.mult
                    )
                    # sum over candidates, keeping the (u, v) pair split: the
                    # concat layout viewed [P, 2, 128] reduces innermost to
                    # comb[:, 2k] = u_k, comb[:, 2k+1] = v_k
                    nc.vector.tensor_reduce(
                        out=comb[:, 2 * k:2 * k + 2],
                        in_=prod[:].rearrange("p (c j) -> p c j", c=2),
                        axis=AX.X, op=OP.add,
                    )

                # w = softmax(d2_topk / tau); cvals8[:, :K] = -d2 (descending),
                # so the softmax argmax is at column K-1.
                nrmaxh = small.tile([P, 1], f32)
                nc.vector.tensor_scalar(
                    out=nrmaxh[:], in0=cvals8[:, K - 1:K], scalar1=float(0.5 / TAU),
                    scalar2=None, op0=OP.mult,
                )
                # exp(x) = 2/(1 - tanh(x/2)) - 1 with x = -(vals - vals_min)/tau;
                # keeps ACT inside the gelu_and_others table set (no exp there).
                th = small.tile([P, K], f32)
                nc.scalar.activation(
                    th[:], cvals8[:, 0:K], AF.Tanh,
                    bias=nrmaxh[:, 0:1], scale=float(-0.5 / TAU),
                )
                denom = small.tile([P, K], f32)
                nc.vector.tensor_scalar(
                    out=denom[:], in0=th[:], scalar1=-1.0, scalar2=1.0,
                    op0=OP.mult, op1=OP.add,
                )
                rden = small.tile([P, K], f32)
                nc.vector.reciprocal(rden[:], denom[:])
                e4 = small.tile([P, K], f32)
                nc.vector.tensor_scalar(
                    out=e4[:], in0=rden[:], scalar1=2.0, scalar2=-1.0,
                    op0=OP.mult, op1=OP.add,
                )
                ssum = small.tile([P, 1], f32)
                nc.vector.reduce_sum(out=ssum[:], in_=e4[:], axis=AX.X)
                rinv = small.tile([P, 1], f32)
                nc.vector.reciprocal(rinv[:], ssum[:])
                wnorm = small.tile([P, K], f32)
                nc.vector.tensor_scalar(
                    out=wnorm[:], in0=e4[:], scalar1=rinv[:, 0:1],
                    scalar2=None, op0=OP.mult,
                )

                # [P, 2] slices -> [2, P] each, packed into [2, K*P]:
                # columns k*P..(k+1)*P hold (Ax, Ay) of neighbor k for all b
                tp_ps = psum.tile([2, K * P], f32)
                for k in range(K):
                    nc.tensor.transpose(
                        out=tp_ps[:, k * P:(k + 1) * P],
                        in_=comb[:, 2 * k:2 * k + 2], identity=ident[:],
                    )
                rhs_all = mlp.tile([2, K * P], f32)
                nc.vector.tensor_copy(rhs_all[:], tp_ps[:])

                hp = psum.tile([EMB, K * P], f32)
                nc.tensor.matmul(
                    out=hp[:], lhsT=w1t[:], rhs=rhs_all[:], start=True, stop=True
                )
                h1 = mlp.tile([EMB, K * P], f32)
                nc.scalar.activation(h1[:], hp[:], AF.Gelu, bias=b1c[:, 0:1], scale=1.0)
                h2p = psum.tile([EMB, K * P], f32)
                nc.tensor.matmul(
                    out=h2p[:], lhsT=w2t[:], rhs=h1[:], start=True, stop=True
                )
                h2 = mlp.tile([EMB, K * P], f32)
                nc.scalar.activation(h2[:], h2p[:], AF.Gelu, bias=b2c[:, 0:1], scale=1.0)

                # weighted sum over neighbors, back in b-on-partitions layout.
                osb = small.tile([P, EMB], f32)
                nc.vector.memset(osb[:], 0.0)
                for k in range(K):
                    h2tp = psum.tile([P, EMB], f32)
                    nc.tensor.transpose(
                        out=h2tp[:], in_=h2[:, k * P:(k + 1) * P],
                        identity=ident[0:EMB, 0:EMB],
                    )
                    nc.vector.scalar_tensor_tensor(
                        out=osb[:], in0=h2tp[:], scalar=wnorm[:, k:k + 1],
                        in1=osb[:], op0=OP.mult, op1=OP.add,
                    )
                nc.scalar.dma_start(
                    out=out_h[rows, br * EMB:(br + 1) * EMB], in_=osb[:]
                )
                if debug:
                    dbf = small.tile([P, 32], f32)
                    nc.vector.tensor_copy(dbf[:, 0:8], cvals8[:])
                    nc.vector.tensor_copy(dbf[:, 8:16], comb[:])
                    nc.vector.tensor_copy(dbf[:, 16:20], wnorm[:])
                    nc.vector.tensor_copy(dbf[:, 20:28], svals8[:])
                    nc.sync.dma_start(out=dbgf_h[rows, br, :], in_=dbf[:])
                    dbi = small.tile([P, 16], u32)
                    nc.vector.tensor_copy(dbi[:, 0:8], sidx8[:])
                    nc.vector.tensor_copy(dbi[:, 8:16], cpos8[:])
                    nc.sync.dma_start(out=dbgi_h[rows, br, :], in_=dbi[:])
    return nc


def _get_nc(debug=False, variant=""):
    key = ("nc", debug, variant)
    if key not in _CACHE:
        nc = _build(debug, variant)
        nc.finalize()  # runs the Bacc passes (event sems, reg alloc, table loads)
        _CACHE[key] = nc
    return _CACHE[key]


def _make_in_maps(inputs):
    nodes = np.asarray(inputs["nodes_2x2"], dtype=np.float32)
    ancS = np.asarray(inputs["ancS"], dtype=np.float32)
    ancL = np.asarray(inputs["ancL"], dtype=np.float32)
    W1 = np.asarray(inputs["W1"], dtype=np.float32)
    b1 = np.asarray(inputs["b1"], dtype=np.float32)
    W2 = np.asarray(inputs["W2"], dtype=np.float32)
    b2 = np.asarray(inputs["b2"], dtype=np.float32)
    in_maps = []
    for c in range(NCORES):
        sl = slice(c * BLOC, (c + 1) * BLOC)
        in_maps.append(
            {
                "nodes": np.ascontiguousarray(nodes[sl]),
                "ancS": np.ascontiguousarray(ancS[sl]),
                "ancL": np.ascontiguousarray(ancL[sl]),
                "W1": W1,
                "b1": b1,
                "W2": W2,
                "b2": b2,
            }
        )
    return in_maps


def _run(in_maps, trace=False, debug=False, variant=""):
    from concourse.bass_utils import run_bass_kernel_spmd

    nc = _get_nc(debug, variant)
    return run_bass_kernel_spmd(nc, in_maps, core_ids=list(range(NCORES)), trace=trace)


def kernel(**inputs):
    in_maps = _make_in_maps(inputs)
    res = _run(in_maps).results
    out = np.concatenate([res[c]["out"] for c in range(NCORES)], axis=0)
    return out[:, :EMB].copy(), out[:, EMB:].copy()
